# revision 4
# baseline (speedup 1.0000x reference)
"""Trainium2 SPMD kernel for nn_Attentionlayer_9208409883387.

Mathematical simplification: the reference computes
    h   = x @ W
    att = softmax(mask(leaky_relu(s1+s2), adj), axis=3)
    res = leaky_relu(h * sum_j att[..., j])
The row-sum of a softmax along its normalization axis is identically 1
(every row has >=1 unmasked entry: P[all-zero adj row] ~ 2^-1024), so
    res = leaky_relu(x @ W)
exactly, up to fp32 rounding of the softmax row-sum.

Strategy (v2, bf16): data-parallel over the 48*1024 = 49152 rows, 6144
rows/core.  Each core's shard is packed host-side in bf16 with f_in on
partitions (xpack[0:64] = rows[0:3072].T, xpack[64:128] = rows[3072:].T),
W replicated as a block-diagonal [128,128] bf16 stationary operand.  The
kernel is HBM-bound; bf16 halves both streams vs the fp32 v1 (in 819KB +
out 786KB per core).  The two HWDGE rings are load-balanced: the sync
(SP) ring streams W+chunks 0-2 in and y0-y2 out; the scalar (ACT) ring
streams chunks 3-5 in and y3-y5 out.  leaky_relu runs on the DVE as a
single scalar_tensor_tensor max(ps*0.01, ps) reading PSUM, writing bf16
(no ACT table load needed).  Host upcasts the bf16 output to fp32
(measured rel L2 error ~2e-3 vs the fp32 reference, gate is 2e-2).
"""

import numpy as np

B, T, N, F = 4, 12, 1024, 64
N_CORES = 8
ROWS = B * T * N              # 49152
RPC = ROWS // N_CORES         # 6144 rows per core
HALF = RPC // 2               # 3072 packed columns per core
CHUNK = 512                   # one PSUM bank of fp32 accumulators
NCHUNK = HALF // CHUNK        # 6

_PROGRAM = None

# Compute/lrelu order = chunk arrival order (rings alternate).
ORDER = [0, 3, 1, 4, 2, 5]


def _build_program():
    """Raw-Bass pipeline, bf16 in/out, both HWDGE rings balanced.

    sync  (SP ring):  in W+c0 | c1 | c2,  out y0y1 | y2, final quiesce
    scalar(ACT ring): in c3 | c4 | c5,    out y3y4 | y5
    tensor: one bf16 matmul per chunk (block-diag W stationary)
    vector: lrelu = max(ps*0.01, ps), PSUM -> SBUF bf16
    """
    import concourse.bass as bass
    import concourse.mybir as mybir
    from contextlib import ExitStack

    bf16 = mybir.dt.bfloat16
    f32 = mybir.dt.float32
    nc = bass.Bass("TRN2")
    xp = nc.declare_dram_parameter("xpack", [128, 128 + HALF], bf16, isOutput=False)
    yp = nc.declare_dram_parameter("ypack", [128, HALF], bf16, isOutput=True)

    def xcol(i):  # xpack column range of chunk i
        return 128 + i * CHUNK, 128 + (i + 1) * CHUNK

    with ExitStack() as ctx:
        x_sb = ctx.enter_context(nc.sbuf_tensor("x_sb", [128, 128 + HALF], bf16))
        y_sb = ctx.enter_context(nc.sbuf_tensor("y_sb", [128, HALF], bf16))
        tmp_sb = ctx.enter_context(nc.sbuf_tensor("tmp_sb", [128, CHUNK], bf16))
        ps = [
            ctx.enter_context(nc.psum_tensor(f"ps{i}", [128, CHUNK], f32))
            for i in range(NCHUNK)
        ]
        # One semaphore per input DMA (sub-completions of different
        # transfers must not share a counter).
        in_sems = [ctx.enter_context(nc.semaphore(f"din{i}")) for i in range(NCHUNK)]
        pe_sem = ctx.enter_context(nc.semaphore("pe_sem"))
        v_sem = ctx.enter_context(nc.semaphore("v_sem"))
        out_sem = ctx.enter_context(nc.semaphore("out_sem"))
        block = ctx.enter_context(nc.Block())

        # v_sem counts lrelu completions in ORDER; count needed for chunk i:
        vneed = {ci: k + 1 for k, ci in enumerate(ORDER)}

        @block.sync
        def _(sync):
            # in: W rides with chunk 0
            sync.dma_start(out=x_sb[:, 0:640], in_=xp[:, 0:640]).then_inc(
                in_sems[0], 16
            )
            for i in (1, 2):
                lo, hi = xcol(i)
                sync.dma_start(out=x_sb[:, lo:hi], in_=xp[:, lo:hi]).then_inc(
                    in_sems[i], 16
                )
            # out: y0+y1, then y2
            sync.wait_ge(v_sem, max(vneed[0], vneed[1]))
            sync.dma_start(out=yp[:, 0:1024], in_=y_sb[:, 0:1024]).then_inc(
                out_sem, 16
            )
            sync.wait_ge(v_sem, vneed[2])
            sync.dma_start(out=yp[:, 1024:1536], in_=y_sb[:, 1024:1536]).then_inc(
                out_sem, 16
            )
            # quiesce: all 4 out DMAs (both rings) complete before teardown
            sync.wait_ge(out_sem, 64)

        @block.scalar
        def _(scalar):
            for i in (3, 4, 5):
                lo, hi = xcol(i)
                scalar.dma_start(out=x_sb[:, lo:hi], in_=xp[:, lo:hi]).then_inc(
                    in_sems[i], 16
                )
            scalar.wait_ge(v_sem, max(vneed[3], vneed[4]))
            scalar.dma_start(out=yp[:, 1536:2560], in_=y_sb[:, 1536:2560]).then_inc(
                out_sem, 16
            )
            scalar.wait_ge(v_sem, vneed[5])
            scalar.dma_start(out=yp[:, 2560:3072], in_=y_sb[:, 2560:3072]).then_inc(
                out_sem, 16
            )

        @block.tensor
        def _(tensor):
            w_ap = x_sb[:, 0:128]
            for ci in ORDER:
                tensor.wait_ge(in_sems[ci], 16)
                lo, hi = xcol(ci)
                nc.tensor.matmul(
                    ps[ci][:], w_ap, x_sb[:, lo:hi], start=True, stop=True
                ).then_inc(pe_sem, 1)

        @block.vector
        def _(vector):
            # lrelu(x) = max(x, 0.01x).  A single instruction may read PSUM
            # only once, so: tmp = ps*0.01 (PSUM->SBUF), y = max(ps, tmp).
            for k, ci in enumerate(ORDER):
                vector.wait_ge(pe_sem, k + 1)
                nc.vector.tensor_scalar_mul(tmp_sb[:], ps[ci][:], 0.01)
                nc.vector.tensor_tensor(
                    y_sb[:, ci * CHUNK : (ci + 1) * CHUNK],
                    ps[ci][:],
                    tmp_sb[:],
                    op=mybir.AluOpType.max,
                ).then_inc(v_sem, 1)

    nc.finalize()
    return nc


def _get_program():
    global _PROGRAM
    if _PROGRAM is None:
        _PROGRAM = _build_program()
    return _PROGRAM


def _make_in_maps(x, W):
    import ml_dtypes

    bf16 = ml_dtypes.bfloat16
    xr = np.ascontiguousarray(x, dtype=np.float32).reshape(N_CORES, RPC, F)
    wpack = np.zeros((128, 128), dtype=bf16)
    wb = np.asarray(W, dtype=np.float32).astype(bf16)
    wpack[0:64, 0:64] = wb
    wpack[64:128, 64:128] = wb
    in_maps = []
    for c in range(N_CORES):
        xpack = np.empty((128, 128 + HALF), dtype=bf16)
        xpack[:, 0:128] = wpack
        xpack[0:64, 128:] = xr[c, 0:HALF].T.astype(bf16)
        xpack[64:128, 128:] = xr[c, HALF:].T.astype(bf16)
        in_maps.append({"xpack": xpack})
    return in_maps


def run_spmd(x, W, **spmd_kwargs):
    """Run the Bass program on 8 cores; returns (y_full, BassKernelResults)."""
    from concourse.bass_utils import run_bass_kernel_spmd

    in_maps = _make_in_maps(x, W)
    res = run_bass_kernel_spmd(
        _get_program(), in_maps, list(range(N_CORES)), **spmd_kwargs
    )
    y = np.empty((N_CORES, RPC, F), np.float32)
    for c in range(N_CORES):
        ypack = np.asarray(res.results[c]["ypack"]).astype(np.float32)
        y[c, 0:HALF] = ypack[0:64].T
        y[c, HALF:] = ypack[64:128].T
    return y.reshape(B, T, N, F), res


def kernel(x, adj, W, a):
    # adj and a are mathematically dead (softmax row-sum == 1); see module doc.
    y, _ = run_spmd(np.asarray(x), np.asarray(W, dtype=np.float32))
    return y


# revision 13
# speedup vs baseline: 1.1251x; 1.1251x over previous
"""Trainium2 SPMD kernel for nn_Attentionlayer_9208409883387.

Mathematical simplification: the reference computes
    h   = x @ W
    att = softmax(mask(leaky_relu(s1+s2), adj), axis=3)
    res = leaky_relu(h * sum_j att[..., j])
The row-sum of a softmax along its normalization axis is identically 1
(every row has >=1 unmasked entry: P[all-zero adj row] ~ 2^-1024), so
    res = leaky_relu(x @ W)
exactly, up to fp32 rounding of the softmax row-sum.

Strategy (v3, bf16 + quadrant-tiled PE): data-parallel over the
48*1024 = 49152 rows, 6144 rows/core.  Each core's shard is packed
host-side in bf16 with f_in on partitions (xpack[0:64] = rows[0:3072].T,
xpack[64:128] = rows[3072:].T), W replicated block-diagonally so both
64x64 diagonal blocks hold W.  Per 512-col chunk the PE runs TWO 64x64
half-matmuls on quadrant tiles (0,0) and (64,64) which execute
concurrently (independent row/col groups).  DMA transfers are sized
>=2KB per partition line (small-descriptor bandwidth collapse measured
at 1KB).  leaky_relu is split across three engines: ACT (Lrelu spans
over 2 PSUM banks), DVE and GpSimd (tmp=ps*0.01; max(ps,tmp)).  Ring
balance: sync/SP ring streams W+c0+c1 in and y0..y3 out; scalar/ACT
ring streams c2..c5 in and y4,y5 out.  Host upcasts bf16 output.
"""

import numpy as np

B, T, N, F = 4, 12, 1024, 64
N_CORES = 8
ROWS = B * T * N              # 49152
RPC = ROWS // N_CORES         # 6144 rows per core
HALF = RPC // 2               # 3072 packed columns per core
CHUNK = 512                   # one PSUM bank of fp32 accumulators
NCHUNK = HALF // CHUNK        # 6

_PROGRAM = None


def _build_program():
    import concourse.bass as bass
    import concourse.mybir as mybir
    from contextlib import ExitStack

    bf16 = mybir.dt.bfloat16
    f32 = mybir.dt.float32
    nc = bass.Bass("TRN2")
    xp = nc.declare_dram_parameter("xpack", [128, 128 + HALF], bf16, isOutput=False)
    yp = nc.declare_dram_parameter("ypack", [128, HALF], bf16, isOutput=True)

    def xc(i):  # xpack column range of chunk i
        return 128 + i * CHUNK, 128 + (i + 1) * CHUNK

    with ExitStack() as ctx:
        x_sb = ctx.enter_context(nc.sbuf_tensor("x_sb", [128, 128 + HALF], bf16))
        y_sb = ctx.enter_context(nc.sbuf_tensor("y_sb", [128, HALF], bf16))
        tmpD = ctx.enter_context(nc.sbuf_tensor("tmpD", [128, CHUNK], bf16))
        # banks 0-1 and 2-3 as 2-bank tensors so one ACT Lrelu spans both
        ps01 = ctx.enter_context(nc.psum_tensor("ps01", [128, 2 * CHUNK], f32))
        ps23 = ctx.enter_context(nc.psum_tensor("ps23", [128, 2 * CHUNK], f32))
        ps4 = ctx.enter_context(nc.psum_tensor("ps4", [128, CHUNK], f32))
        ps5 = ctx.enter_context(nc.psum_tensor("ps5", [128, CHUNK], f32))
        # per-chunk matmul output slices (each within a single PSUM bank)
        psc = [
            ps01[:, 0:CHUNK], ps01[:, CHUNK : 2 * CHUNK],
            ps23[:, 0:CHUNK], ps23[:, CHUNK : 2 * CHUNK],
            ps4[:], ps5[:],
        ]
        sA = ctx.enter_context(nc.semaphore("sA"))    # in: W+c0+c1 (SP ring)
        sB1 = ctx.enter_context(nc.semaphore("sB1"))  # in: c2+c3 (ACT ring)
        sB2 = ctx.enter_context(nc.semaphore("sB2"))  # in: c4+c5 (ACT ring)
        pe_sem = ctx.enter_context(nc.semaphore("pe_sem"))  # +1 per finished chunk
        vA = ctx.enter_context(nc.semaphore("vA"))    # ACT lrelu spans done
        vD = ctx.enter_context(nc.semaphore("vD"))    # DVE lrelu bank4 done
        out_sem = ctx.enter_context(nc.semaphore("out_sem"))
        block = ctx.enter_context(nc.Block())

        @block.sync
        def _(sync):
            # in: W + chunks 0,1 -> 1152 cols, 2304B/partition descriptors
            sync.dma_start(out=x_sb[:, 0:1152], in_=xp[:, 0:1152]).then_inc(sA, 16)
            # outs for banks 0..3 on the SP ring
            sync.wait_ge(vA, 1)
            sync.dma_start(out=yp[:, 0:1024], in_=y_sb[:, 0:1024]).then_inc(
                out_sem, 16
            )
            sync.wait_ge(vA, 2)
            sync.dma_start(out=yp[:, 1024:2048], in_=y_sb[:, 1024:2048]).then_inc(
                out_sem, 16
            )
            # all 3 out DMAs (both rings) complete before teardown
            sync.wait_ge(out_sem, 48)

        @block.scalar
        def _(scalar):
            # in: chunks 2,3 then 4,5 -> 1024 cols each, 2048B/partition
            scalar.dma_start(out=x_sb[:, 1152:2176], in_=xp[:, 1152:2176]).then_inc(
                sB1, 16
            )
            scalar.dma_start(out=x_sb[:, 2176:3200], in_=xp[:, 2176:3200]).then_inc(
                sB2, 16
            )
            # Touch the Lrelu table so the lazy ACT_TABLE_LOAD (~1.3us)
            # happens during the DMA in-stream, not before the first span.
            nc.scalar.activation(
                tmpD[0:1, 0:4], tmpD[0:1, 0:4],
                mybir.ActivationFunctionType.Lrelu, alpha=0.01,
            )
            # lrelu spans over PSUM banks 0-1 and 2-3 (2-bank tensors)
            scalar.wait_ge(pe_sem, 2)
            nc.scalar.activation(
                y_sb[:, 0:1024], ps01[:],
                mybir.ActivationFunctionType.Lrelu, alpha=0.01,
            ).then_inc(vA, 1)
            scalar.wait_ge(pe_sem, 4)
            nc.scalar.activation(
                y_sb[:, 1024:2048], ps23[:],
                mybir.ActivationFunctionType.Lrelu, alpha=0.01,
            ).then_inc(vA, 1)
            # bank 5 lrelu (GpSimd cannot access PSUM; DVE has bank 4)
            scalar.wait_ge(pe_sem, 6)
            nc.scalar.activation(
                y_sb[:, 2560:3072], ps5[:],
                mybir.ActivationFunctionType.Lrelu, alpha=0.01,
            )
            # out for banks 4,5 on the ACT ring
            scalar.wait_ge(vD, 1)
            scalar.dma_start(out=yp[:, 2048:3072], in_=y_sb[:, 2048:3072]).then_inc(
                out_sem, 16
            )

        @block.tensor
        def _(tensor):
            w0 = x_sb[0:64, 0:64]        # W in quadrant rows 0:64
            w1 = x_sb[64:128, 64:128]    # W copy in quadrant rows 64:128
            waits = {0: sA, 2: sB1, 4: sB2}
            for ci in range(NCHUNK):
                if ci in waits:
                    tensor.wait_ge(waits[ci], 16)
                lo, hi = xc(ci)
                # two concurrent 64x64 quadrant matmuls per chunk
                nc.tensor.matmul(
                    psc[ci][0:64, :], w0, x_sb[0:64, lo:hi],
                    start=True, stop=True, tile_position=(0, 0),
                )
                nc.tensor.matmul(
                    psc[ci][64:128, :], w1, x_sb[64:128, lo:hi],
                    start=True, stop=True, tile_position=(64, 64),
                ).then_inc(pe_sem, 1)

        @block.vector
        def _(vector):
            # bank 4: tmp = ps*0.01 ; y = max(ps, tmp)
            vector.wait_ge(pe_sem, 5)
            nc.vector.tensor_scalar_mul(tmpD[:], ps4[:], 0.01)
            nc.vector.tensor_tensor(
                y_sb[:, 2048:2560], ps4[:], tmpD[:], op=mybir.AluOpType.max
            ).then_inc(vD, 1)

    nc.finalize()
    return nc


def _get_program():
    global _PROGRAM
    if _PROGRAM is None:
        _PROGRAM = _build_program()
    return _PROGRAM


def _make_in_maps(x, W):
    import ml_dtypes

    bf16 = ml_dtypes.bfloat16
    xr = np.ascontiguousarray(x, dtype=np.float32).reshape(N_CORES, RPC, F)
    wpack = np.zeros((128, 128), dtype=bf16)
    wb = np.asarray(W, dtype=np.float32).astype(bf16)
    wpack[0:64, 0:64] = wb
    wpack[64:128, 64:128] = wb
    in_maps = []
    for c in range(N_CORES):
        xpack = np.empty((128, 128 + HALF), dtype=bf16)
        xpack[:, 0:128] = wpack
        xpack[0:64, 128:] = xr[c, 0:HALF].T.astype(bf16)
        xpack[64:128, 128:] = xr[c, HALF:].T.astype(bf16)
        in_maps.append({"xpack": xpack})
    return in_maps


def run_spmd(x, W, **spmd_kwargs):
    """Run the Bass program on 8 cores; returns (y_full, BassKernelResults)."""
    from concourse.bass_utils import run_bass_kernel_spmd

    in_maps = _make_in_maps(x, W)
    res = run_bass_kernel_spmd(
        _get_program(), in_maps, list(range(N_CORES)), **spmd_kwargs
    )
    y = np.empty((N_CORES, RPC, F), np.float32)
    for c in range(N_CORES):
        ypack = np.asarray(res.results[c]["ypack"]).astype(np.float32)
        y[c, 0:HALF] = ypack[0:64].T
        y[c, HALF:] = ypack[64:128].T
    return y.reshape(B, T, N, F), res


def kernel(x, adj, W, a):
    # adj and a are mathematically dead (softmax row-sum == 1); see module doc.
    y, _ = run_spmd(np.asarray(x), np.asarray(W, dtype=np.float32))
    return y


# revision 16
# speedup vs baseline: 1.1369x; 1.0105x over previous
"""Trainium2 SPMD kernel for nn_Attentionlayer_9208409883387.

Mathematical simplification: the reference computes
    h   = x @ W
    att = softmax(mask(leaky_relu(s1+s2), adj), axis=3)
    res = leaky_relu(h * sum_j att[..., j])
The row-sum of a softmax along its normalization axis is identically 1
(every row has >=1 unmasked entry: P[all-zero adj row] ~ 2^-1024), so
    res = leaky_relu(x @ W)
exactly, up to fp32 rounding of the softmax row-sum.

Strategy (v3, bf16 + quadrant-tiled PE): data-parallel over the
48*1024 = 49152 rows, 6144 rows/core.  Each core's shard is packed
host-side in bf16 with f_in on partitions (xpack[0:64] = rows[0:3072].T,
xpack[64:128] = rows[3072:].T), W replicated block-diagonally so both
64x64 diagonal blocks hold W.  Per 512-col chunk the PE runs TWO 64x64
half-matmuls on quadrant tiles (0,0) and (64,64) which execute
concurrently (independent row/col groups).  DMA transfers are sized
>=2KB per partition line (small-descriptor bandwidth collapse measured
at 1KB).  leaky_relu is split across three engines: ACT (Lrelu spans
over 2 PSUM banks), DVE and GpSimd (tmp=ps*0.01; max(ps,tmp)).  Ring
balance: sync/SP ring streams W+c0+c1 in and y0..y3 out; scalar/ACT
ring streams c2..c5 in and y4,y5 out.  Host upcasts bf16 output.
"""

import numpy as np

B, T, N, F = 4, 12, 1024, 64
N_CORES = 8
ROWS = B * T * N              # 49152
RPC = ROWS // N_CORES         # 6144 rows per core
HALF = RPC // 2               # 3072 packed columns per core
CHUNK = 512                   # one PSUM bank of fp32 accumulators
NCHUNK = HALF // CHUNK        # 6

_PROGRAM = None


def _build_program():
    import concourse.bass as bass
    import concourse.mybir as mybir
    from contextlib import ExitStack

    bf16 = mybir.dt.bfloat16
    f32 = mybir.dt.float32
    nc = bass.Bass("TRN2")
    xp = nc.declare_dram_parameter("xpack", [128, 128 + HALF], bf16, isOutput=False)
    yp = nc.declare_dram_parameter("ypack", [128, HALF], bf16, isOutput=True)

    def xc(i):  # xpack column range of chunk i
        return 128 + i * CHUNK, 128 + (i + 1) * CHUNK

    with ExitStack() as ctx:
        x_sb = ctx.enter_context(nc.sbuf_tensor("x_sb", [128, 128 + HALF], bf16))
        y_sb = ctx.enter_context(nc.sbuf_tensor("y_sb", [128, HALF], bf16))
        tmpD = ctx.enter_context(nc.sbuf_tensor("tmpD", [128, CHUNK], bf16))
        # banks 0-1 and 2-3 as 2-bank tensors so one ACT Lrelu spans both
        ps01 = ctx.enter_context(nc.psum_tensor("ps01", [128, 2 * CHUNK], f32))
        ps23 = ctx.enter_context(nc.psum_tensor("ps23", [128, 2 * CHUNK], f32))
        ps4 = ctx.enter_context(nc.psum_tensor("ps4", [128, CHUNK], f32))
        ps5 = ctx.enter_context(nc.psum_tensor("ps5", [128, CHUNK], f32))
        # per-chunk matmul output slices (each within a single PSUM bank)
        psc = [
            ps01[:, 0:CHUNK], ps01[:, CHUNK : 2 * CHUNK],
            ps23[:, 0:CHUNK], ps23[:, CHUNK : 2 * CHUNK],
            ps4[:], ps5[:],
        ]
        sA = ctx.enter_context(nc.semaphore("sA"))    # in: W+c0 (SP ring)
        sA1 = ctx.enter_context(nc.semaphore("sA1"))  # in: c1 (SP ring)
        sB1 = ctx.enter_context(nc.semaphore("sB1"))  # in: c2+c3 (ACT ring)
        sB2 = ctx.enter_context(nc.semaphore("sB2"))  # in: c4+c5 (ACT ring)
        pe_sem = ctx.enter_context(nc.semaphore("pe_sem"))  # +1 per finished chunk
        vA = ctx.enter_context(nc.semaphore("vA"))    # ACT lrelu spans done
        vD = ctx.enter_context(nc.semaphore("vD"))    # DVE lrelu bank4 done
        out_sem = ctx.enter_context(nc.semaphore("out_sem"))
        block = ctx.enter_context(nc.Block())

        @block.sync
        def _(sync):
            # in: W+c0 first (smaller -> earlier PE start), then c1
            sync.dma_start(out=x_sb[:, 0:640], in_=xp[:, 0:640]).then_inc(sA, 16)
            sync.dma_start(out=x_sb[:, 640:1152], in_=xp[:, 640:1152]).then_inc(
                sA1, 16
            )
            # outs for banks 0..3 on the SP ring
            sync.wait_ge(vA, 1)
            sync.dma_start(out=yp[:, 0:1024], in_=y_sb[:, 0:1024]).then_inc(
                out_sem, 16
            )
            sync.wait_ge(vA, 2)
            sync.dma_start(out=yp[:, 1024:2048], in_=y_sb[:, 1024:2048]).then_inc(
                out_sem, 16
            )
            # all 3 out DMAs (both rings) complete before teardown
            sync.wait_ge(out_sem, 48)

        @block.scalar
        def _(scalar):
            # in: chunks 2,3 then 4,5 -> 1024 cols each, 2048B/partition
            scalar.dma_start(out=x_sb[:, 1152:2176], in_=xp[:, 1152:2176]).then_inc(
                sB1, 16
            )
            scalar.dma_start(out=x_sb[:, 2176:3200], in_=xp[:, 2176:3200]).then_inc(
                sB2, 16
            )
            # Touch the Lrelu table so the lazy ACT_TABLE_LOAD (~1.3us)
            # happens during the DMA in-stream, not before the first span.
            nc.scalar.activation(
                tmpD[0:1, 0:4], tmpD[0:1, 0:4],
                mybir.ActivationFunctionType.Lrelu, alpha=0.01,
            )
            # lrelu spans over PSUM banks 0-1 and 2-3 (2-bank tensors)
            scalar.wait_ge(pe_sem, 2)
            nc.scalar.activation(
                y_sb[:, 0:1024], ps01[:],
                mybir.ActivationFunctionType.Lrelu, alpha=0.01,
            ).then_inc(vA, 1)
            scalar.wait_ge(pe_sem, 4)
            nc.scalar.activation(
                y_sb[:, 1024:2048], ps23[:],
                mybir.ActivationFunctionType.Lrelu, alpha=0.01,
            ).then_inc(vA, 1)
            # bank 5 lrelu (GpSimd cannot access PSUM; DVE has bank 4)
            scalar.wait_ge(pe_sem, 6)
            nc.scalar.activation(
                y_sb[:, 2560:3072], ps5[:],
                mybir.ActivationFunctionType.Lrelu, alpha=0.01,
            )
            # out for banks 4,5 on the ACT ring
            scalar.wait_ge(vD, 1)
            scalar.dma_start(out=yp[:, 2048:3072], in_=y_sb[:, 2048:3072]).then_inc(
                out_sem, 16
            )

        @block.tensor
        def _(tensor):
            # W output columns split 32-wide -> four concurrent PE tiles
            # per chunk: (0,0) (0,32) for row-block 1, (64,64) (64,96) for
            # row-block 2.
            w00 = x_sb[0:64, 0:32]
            w01 = x_sb[0:64, 32:64]
            w10 = x_sb[64:128, 64:96]
            w11 = x_sb[64:128, 96:128]
            waits = {0: sA, 1: sA1, 2: sB1, 4: sB2}
            for ci in range(NCHUNK):
                if ci in waits:
                    tensor.wait_ge(waits[ci], 16)
                lo, hi = xc(ci)
                mv0 = x_sb[0:64, lo:hi]
                mv1 = x_sb[64:128, lo:hi]
                nc.tensor.matmul(
                    psc[ci][0:32, :], w00, mv0,
                    start=True, stop=True, tile_position=(0, 0),
                )
                nc.tensor.matmul(
                    psc[ci][32:64, :], w01, mv0,
                    start=True, stop=True, tile_position=(0, 32),
                )
                nc.tensor.matmul(
                    psc[ci][64:96, :], w10, mv1,
                    start=True, stop=True, tile_position=(64, 64),
                )
                nc.tensor.matmul(
                    psc[ci][96:128, :], w11, mv1,
                    start=True, stop=True, tile_position=(64, 96),
                ).then_inc(pe_sem, 1)

        @block.vector
        def _(vector):
            # bank 4: tmp = ps*0.01 ; y = max(ps, tmp)
            vector.wait_ge(pe_sem, 5)
            nc.vector.tensor_scalar_mul(tmpD[:], ps4[:], 0.01)
            nc.vector.tensor_tensor(
                y_sb[:, 2048:2560], ps4[:], tmpD[:], op=mybir.AluOpType.max
            ).then_inc(vD, 1)

    nc.finalize()
    return nc


def _get_program():
    global _PROGRAM
    if _PROGRAM is None:
        _PROGRAM = _build_program()
    return _PROGRAM


def _make_in_maps(x, W):
    import ml_dtypes

    bf16 = ml_dtypes.bfloat16
    xr = np.ascontiguousarray(x, dtype=np.float32).reshape(N_CORES, RPC, F)
    wpack = np.zeros((128, 128), dtype=bf16)
    wb = np.asarray(W, dtype=np.float32).astype(bf16)
    wpack[0:64, 0:64] = wb
    wpack[64:128, 64:128] = wb
    in_maps = []
    for c in range(N_CORES):
        xpack = np.empty((128, 128 + HALF), dtype=bf16)
        xpack[:, 0:128] = wpack
        xpack[0:64, 128:] = xr[c, 0:HALF].T.astype(bf16)
        xpack[64:128, 128:] = xr[c, HALF:].T.astype(bf16)
        in_maps.append({"xpack": xpack})
    return in_maps


def run_spmd(x, W, **spmd_kwargs):
    """Run the Bass program on 8 cores; returns (y_full, BassKernelResults)."""
    from concourse.bass_utils import run_bass_kernel_spmd

    in_maps = _make_in_maps(x, W)
    res = run_bass_kernel_spmd(
        _get_program(), in_maps, list(range(N_CORES)), **spmd_kwargs
    )
    y = np.empty((N_CORES, RPC, F), np.float32)
    for c in range(N_CORES):
        ypack = np.asarray(res.results[c]["ypack"]).astype(np.float32)
        y[c, 0:HALF] = ypack[0:64].T
        y[c, HALF:] = ypack[64:128].T
    return y.reshape(B, T, N, F), res


def kernel(x, adj, W, a):
    # adj and a are mathematically dead (softmax row-sum == 1); see module doc.
    y, _ = run_spmd(np.asarray(x), np.asarray(W, dtype=np.float32))
    return y


# revision 17
# speedup vs baseline: 1.1462x; 1.0082x over previous
"""Trainium2 SPMD kernel for nn_Attentionlayer_9208409883387.

Mathematical simplification: the reference computes
    h   = x @ W
    att = softmax(mask(leaky_relu(s1+s2), adj), axis=3)
    res = leaky_relu(h * sum_j att[..., j])
The row-sum of a softmax along its normalization axis is identically 1
(every row has >=1 unmasked entry: P[all-zero adj row] ~ 2^-1024), so
    res = leaky_relu(x @ W)
exactly, up to fp32 rounding of the softmax row-sum.

Strategy (v5, bf16): data-parallel over 48*1024 = 49152 rows, 6144
rows/core, packed host-side in bf16 with f_in on partitions
(xpack[0:64] = rows[0:3072].T, xpack[64:128] = rows[3072:].T) and W
replicated block-diagonally.  The first input transfer (W+c0+c1) is
split by partition halves across BOTH HWDGE rings — disjoint SDMA
engine sets move the halves concurrently, halving time-to-first-matmul.
Remaining input and all outputs use >=2KB-per-partition descriptors
(1KB descriptors measured ~2x slower).  Per 512-col chunk the PE runs
two 64x64 quadrant matmuls at tile positions (0,0)/(64,64) which
execute concurrently.  leaky_relu: ACT engine Lrelu over 2-bank PSUM
spans (banks 0-1, 2-3) + single bank 5; DVE covers bank 4 with
tmp=ps*0.01; max(ps,tmp).  Teardown (~7us walrus semaphore-reset
ladder + barriers) and preamble (~7us) are toolchain-fixed; the kernel
minimizes the last-output-receipt time which gates them.
"""

import numpy as np

B, T, N, F = 4, 12, 1024, 64
N_CORES = 8
ROWS = B * T * N              # 49152
RPC = ROWS // N_CORES         # 6144 rows per core
HALF = RPC // 2               # 3072 packed columns per core
CHUNK = 512                   # one PSUM bank of fp32 accumulators
NCHUNK = HALF // CHUNK        # 6

_PROGRAM = None


def _build_program():
    import concourse.bass as bass
    import concourse.mybir as mybir
    from contextlib import ExitStack

    bf16 = mybir.dt.bfloat16
    f32 = mybir.dt.float32
    nc = bass.Bass("TRN2")
    xp = nc.declare_dram_parameter("xpack", [128, 128 + HALF], bf16, isOutput=False)
    yp = nc.declare_dram_parameter("ypack", [128, HALF], bf16, isOutput=True)

    def xc(i):  # xpack column range of chunk i
        return 128 + i * CHUNK, 128 + (i + 1) * CHUNK

    with ExitStack() as ctx:
        x_sb = ctx.enter_context(nc.sbuf_tensor("x_sb", [128, 128 + HALF], bf16))
        y_sb = ctx.enter_context(nc.sbuf_tensor("y_sb", [128, HALF], bf16))
        tmpD = ctx.enter_context(nc.sbuf_tensor("tmpD", [128, CHUNK], bf16))
        # banks 0-1 and 2-3 as 2-bank tensors so one ACT Lrelu spans both
        ps01 = ctx.enter_context(nc.psum_tensor("ps01", [128, 2 * CHUNK], f32))
        ps23 = ctx.enter_context(nc.psum_tensor("ps23", [128, 2 * CHUNK], f32))
        ps4 = ctx.enter_context(nc.psum_tensor("ps4", [128, CHUNK], f32))
        ps5 = ctx.enter_context(nc.psum_tensor("ps5", [128, CHUNK], f32))
        psc = [
            ps01[:, 0:CHUNK], ps01[:, CHUNK : 2 * CHUNK],
            ps23[:, 0:CHUNK], ps23[:, CHUNK : 2 * CHUNK],
            ps4[:], ps5[:],
        ]
        sT = ctx.enter_context(nc.semaphore("sT"))    # in: W+c0+c1 top half
        sBt = ctx.enter_context(nc.semaphore("sBt"))  # in: W+c0+c1 bottom half
        sA1 = ctx.enter_context(nc.semaphore("sA1"))  # in: c2+c3 (SP ring)
        sB2 = ctx.enter_context(nc.semaphore("sB2"))  # in: c4+c5 (ACT ring)
        pe_sem = ctx.enter_context(nc.semaphore("pe_sem"))  # +1 per chunk
        vA = ctx.enter_context(nc.semaphore("vA"))    # ACT lrelu spans done
        vD = ctx.enter_context(nc.semaphore("vD"))    # DVE lrelu bank4 done
        out_sem = ctx.enter_context(nc.semaphore("out_sem"))
        block = ctx.enter_context(nc.Block())

        @block.sync
        def _(sync):
            # first transfer split by partition halves across both rings:
            # disjoint SDMA engine sets -> concurrent flow (2304B/partition)
            sync.dma_start(out=x_sb[0:64, 0:1152], in_=xp[0:64, 0:1152]).then_inc(
                sT, 16
            )
            sync.dma_start(out=x_sb[:, 1152:2176], in_=xp[:, 1152:2176]).then_inc(
                sA1, 16
            )
            sync.wait_ge(vA, 1)
            sync.dma_start(out=yp[:, 0:1024], in_=y_sb[:, 0:1024]).then_inc(
                out_sem, 16
            )
            sync.wait_ge(vA, 2)
            sync.dma_start(out=yp[:, 1024:2048], in_=y_sb[:, 1024:2048]).then_inc(
                out_sem, 16
            )
            # all 3 out DMAs (both rings) complete before teardown
            sync.wait_ge(out_sem, 48)

        @block.scalar
        def _(scalar):
            scalar.dma_start(
                out=x_sb[64:128, 0:1152], in_=xp[64:128, 0:1152]
            ).then_inc(sBt, 16)
            scalar.dma_start(out=x_sb[:, 2176:3200], in_=xp[:, 2176:3200]).then_inc(
                sB2, 16
            )
            # Touch the Lrelu table so the lazy ACT_TABLE_LOAD (~1.3us)
            # happens during the DMA in-stream.
            nc.scalar.activation(
                tmpD[0:1, 0:4], tmpD[0:1, 0:4],
                mybir.ActivationFunctionType.Lrelu, alpha=0.01,
            )
            scalar.wait_ge(pe_sem, 2)
            nc.scalar.activation(
                y_sb[:, 0:1024], ps01[:],
                mybir.ActivationFunctionType.Lrelu, alpha=0.01,
            ).then_inc(vA, 1)
            scalar.wait_ge(pe_sem, 4)
            nc.scalar.activation(
                y_sb[:, 1024:2048], ps23[:],
                mybir.ActivationFunctionType.Lrelu, alpha=0.01,
            ).then_inc(vA, 1)
            # bank 5 lrelu (GpSimd cannot access PSUM; DVE has bank 4)
            scalar.wait_ge(pe_sem, 6)
            nc.scalar.activation(
                y_sb[:, 2560:3072], ps5[:],
                mybir.ActivationFunctionType.Lrelu, alpha=0.01,
            )
            scalar.wait_ge(vD, 1)
            scalar.dma_start(out=yp[:, 2048:3072], in_=y_sb[:, 2048:3072]).then_inc(
                out_sem, 16
            )

        @block.tensor
        def _(tensor):
            w0 = x_sb[0:64, 0:64]        # W in quadrant rows 0:64
            w1 = x_sb[64:128, 64:128]    # W copy in quadrant rows 64:128
            for ci in range(NCHUNK):
                if ci == 0:
                    tensor.wait_ge(sT, 16)
                    tensor.wait_ge(sBt, 16)
                elif ci == 2:
                    tensor.wait_ge(sA1, 16)
                elif ci == 4:
                    tensor.wait_ge(sB2, 16)
                lo, hi = xc(ci)
                nc.tensor.matmul(
                    psc[ci][0:64, :], w0, x_sb[0:64, lo:hi],
                    start=True, stop=True, tile_position=(0, 0),
                )
                nc.tensor.matmul(
                    psc[ci][64:128, :], w1, x_sb[64:128, lo:hi],
                    start=True, stop=True, tile_position=(64, 64),
                ).then_inc(pe_sem, 1)

        @block.vector
        def _(vector):
            # bank 4: tmp = ps*0.01 ; y = max(ps, tmp)
            vector.wait_ge(pe_sem, 5)
            nc.vector.tensor_scalar_mul(tmpD[:], ps4[:], 0.01)
            nc.vector.tensor_tensor(
                y_sb[:, 2048:2560], ps4[:], tmpD[:], op=mybir.AluOpType.max
            ).then_inc(vD, 1)

    nc.finalize()
    return nc


def _get_program():
    global _PROGRAM
    if _PROGRAM is None:
        _PROGRAM = _build_program()
    return _PROGRAM


def _make_in_maps(x, W):
    import ml_dtypes

    bf16 = ml_dtypes.bfloat16
    xr = np.ascontiguousarray(x, dtype=np.float32).reshape(N_CORES, RPC, F)
    wpack = np.zeros((128, 128), dtype=bf16)
    wb = np.asarray(W, dtype=np.float32).astype(bf16)
    wpack[0:64, 0:64] = wb
    wpack[64:128, 64:128] = wb
    in_maps = []
    for c in range(N_CORES):
        xpack = np.empty((128, 128 + HALF), dtype=bf16)
        xpack[:, 0:128] = wpack
        xpack[0:64, 128:] = xr[c, 0:HALF].T.astype(bf16)
        xpack[64:128, 128:] = xr[c, HALF:].T.astype(bf16)
        in_maps.append({"xpack": xpack})
    return in_maps


def run_spmd(x, W, **spmd_kwargs):
    """Run the Bass program on 8 cores; returns (y_full, BassKernelResults)."""
    from concourse.bass_utils import run_bass_kernel_spmd

    in_maps = _make_in_maps(x, W)
    res = run_bass_kernel_spmd(
        _get_program(), in_maps, list(range(N_CORES)), **spmd_kwargs
    )
    y = np.empty((N_CORES, RPC, F), np.float32)
    for c in range(N_CORES):
        ypack = np.asarray(res.results[c]["ypack"]).astype(np.float32)
        y[c, 0:HALF] = ypack[0:64].T
        y[c, HALF:] = ypack[64:128].T
    return y.reshape(B, T, N, F), res


def kernel(x, adj, W, a):
    # adj and a are mathematically dead (softmax row-sum == 1); see module doc.
    y, _ = run_spmd(np.asarray(x), np.asarray(W, dtype=np.float32))
    return y


# revision 22
# speedup vs baseline: 1.1527x; 1.0056x over previous
"""Trainium2 SPMD kernel for nn_Attentionlayer_9208409883387.

Mathematical simplification: the reference computes
    h   = x @ W
    att = softmax(mask(leaky_relu(s1+s2), adj), axis=3)
    res = leaky_relu(h * sum_j att[..., j])
The row-sum of a softmax along its normalization axis is identically 1
(every row has >=1 unmasked entry: P[all-zero adj row] ~ 2^-1024), so
    res = leaky_relu(x @ W)
exactly, up to fp32 rounding of the softmax row-sum.

Strategy (v5, bf16): data-parallel over 48*1024 = 49152 rows, 6144
rows/core, packed host-side in bf16 with f_in on partitions
(xpack[0:64] = rows[0:3072].T, xpack[64:128] = rows[3072:].T) and W
replicated block-diagonally.  The first input transfer (W+c0+c1) is
split by partition halves across BOTH HWDGE rings — disjoint SDMA
engine sets move the halves concurrently, halving time-to-first-matmul.
Remaining input and all outputs use >=2KB-per-partition descriptors
(1KB descriptors measured ~2x slower).  Per 512-col chunk the PE runs
two 64x64 quadrant matmuls at tile positions (0,0)/(64,64) which
execute concurrently.  leaky_relu: ACT engine Lrelu over 2-bank PSUM
spans (banks 0-1, 2-3) + single bank 5; DVE covers bank 4 with
tmp=ps*0.01; max(ps,tmp).  Teardown (~7us walrus semaphore-reset
ladder + barriers) and preamble (~7us) are toolchain-fixed; the kernel
minimizes the last-output-receipt time which gates them.
"""

import numpy as np

B, T, N, F = 4, 12, 1024, 64
N_CORES = 8
ROWS = B * T * N              # 49152
RPC = ROWS // N_CORES         # 6144 rows per core
HALF = RPC // 2               # 3072 packed columns per core
CHUNK = 512                   # one PSUM bank of fp32 accumulators
NCHUNK = HALF // CHUNK        # 6

_PROGRAM = None


def _build_program():
    import concourse.bass as bass
    import concourse.mybir as mybir
    from contextlib import ExitStack

    bf16 = mybir.dt.bfloat16
    f32 = mybir.dt.float32
    nc = bass.Bass("TRN2")
    xp = nc.declare_dram_parameter("xpack", [128, 128 + HALF], bf16, isOutput=False)
    yp = nc.declare_dram_parameter("ypack", [128, HALF], bf16, isOutput=True)

    def xc(i):  # xpack column range of chunk i
        return 128 + i * CHUNK, 128 + (i + 1) * CHUNK

    with ExitStack() as ctx:
        x_sb = ctx.enter_context(nc.sbuf_tensor("x_sb", [128, 128 + HALF], bf16))
        y_sb = ctx.enter_context(nc.sbuf_tensor("y_sb", [128, HALF], bf16))
        tmpD = ctx.enter_context(nc.sbuf_tensor("tmpD", [128, CHUNK], bf16))
        # banks 0-1 and 2-3 as 2-bank tensors so one ACT Lrelu spans both
        ps01 = ctx.enter_context(nc.psum_tensor("ps01", [128, 2 * CHUNK], f32))
        ps23 = ctx.enter_context(nc.psum_tensor("ps23", [128, 2 * CHUNK], f32))
        ps4 = ctx.enter_context(nc.psum_tensor("ps4", [128, CHUNK], f32))
        ps5 = ctx.enter_context(nc.psum_tensor("ps5", [128, CHUNK], f32))
        psc = [
            ps01[:, 0:CHUNK], ps01[:, CHUNK : 2 * CHUNK],
            ps23[:, 0:CHUNK], ps23[:, CHUNK : 2 * CHUNK],
            ps4[:], ps5[:],
        ]
        sT = ctx.enter_context(nc.semaphore("sT"))    # in: W+c0+c1 (SP ring)
        sB1 = ctx.enter_context(nc.semaphore("sB1"))  # in: c2+c3 (ACT ring)
        sG = ctx.enter_context(nc.semaphore("sG"))    # in: c4+c5 (SWDGE)
        pe_sem = ctx.enter_context(nc.semaphore("pe_sem"))  # +1 per chunk
        vA = ctx.enter_context(nc.semaphore("vA"))    # ACT lrelu spans done
        vD = ctx.enter_context(nc.semaphore("vD"))    # DVE lrelu bank4 done
        out_sem = ctx.enter_context(nc.semaphore("out_sem"))
        block = ctx.enter_context(nc.Block())

        @block.sync
        def _(sync):
            # in: W + chunks 0,1 -> 2304B/partition descriptors
            sync.dma_start(out=x_sb[:, 0:1152], in_=xp[:, 0:1152]).then_inc(sT, 16)
            sync.wait_ge(vA, 1)
            sync.dma_start(out=yp[:, 0:1024], in_=y_sb[:, 0:1024]).then_inc(
                out_sem, 16
            )
            sync.wait_ge(vA, 2)
            sync.dma_start(out=yp[:, 1024:2048], in_=y_sb[:, 1024:2048]).then_inc(
                out_sem, 16
            )
            # all 3 out DMAs (both rings) complete before teardown
            sync.wait_ge(out_sem, 48)

        @block.scalar
        def _(scalar):
            # in: chunks 2,3 on the ACT HWDGE ring (2048B/partition)
            scalar.dma_start(out=x_sb[:, 1152:2176], in_=xp[:, 1152:2176]).then_inc(
                sB1, 16
            )
            # Touch the Lrelu table so the lazy ACT_TABLE_LOAD (~1.3us)
            # happens during the DMA in-stream.
            nc.scalar.activation(
                tmpD[0:1, 0:4], tmpD[0:1, 0:4],
                mybir.ActivationFunctionType.Lrelu, alpha=0.01,
            )
            scalar.wait_ge(pe_sem, 2)
            nc.scalar.activation(
                y_sb[:, 0:1024], ps01[:],
                mybir.ActivationFunctionType.Lrelu, alpha=0.01,
            ).then_inc(vA, 1)
            scalar.wait_ge(pe_sem, 4)
            nc.scalar.activation(
                y_sb[:, 1024:2048], ps23[:],
                mybir.ActivationFunctionType.Lrelu, alpha=0.01,
            ).then_inc(vA, 1)
            # bank 5 lrelu (GpSimd cannot access PSUM; DVE has bank 4)
            scalar.wait_ge(pe_sem, 6)
            nc.scalar.activation(
                y_sb[:, 2560:3072], ps5[:],
                mybir.ActivationFunctionType.Lrelu, alpha=0.01,
            )
            scalar.wait_ge(vD, 1)
            scalar.dma_start(out=yp[:, 2048:3072], in_=y_sb[:, 2048:3072]).then_inc(
                out_sem, 16
            )

        @block.tensor
        def _(tensor):
            w0 = x_sb[0:64, 0:64]        # W in quadrant rows 0:64
            w1 = x_sb[64:128, 64:128]    # W copy in quadrant rows 64:128
            waits = {0: sT, 2: sB1, 4: sG}
            for ci in range(NCHUNK):
                if ci in waits:
                    tensor.wait_ge(waits[ci], 16)
                lo, hi = xc(ci)
                nc.tensor.matmul(
                    psc[ci][0:64, :], w0, x_sb[0:64, lo:hi],
                    start=True, stop=True, tile_position=(0, 0),
                )
                nc.tensor.matmul(
                    psc[ci][64:128, :], w1, x_sb[64:128, lo:hi],
                    start=True, stop=True, tile_position=(64, 64),
                ).then_inc(pe_sem, 1)

        @block.vector
        def _(vector):
            # bank 4: tmp = ps*0.01 ; y = max(ps, tmp)
            vector.wait_ge(pe_sem, 5)
            nc.vector.tensor_scalar_mul(tmpD[:], ps4[:], 0.01)
            nc.vector.tensor_tensor(
                y_sb[:, 2048:2560], ps4[:], tmpD[:], op=mybir.AluOpType.max
            ).then_inc(vD, 1)

        @block.gpsimd
        def _(gpsimd):
            # in: chunks 4,5 via the software DGE queue — third parallel
            # DMA path; the GpSimd engine is otherwise idle.
            gpsimd.dma_start(out=x_sb[:, 2176:3200], in_=xp[:, 2176:3200]).then_inc(
                sG, 16
            )

    nc.finalize()
    return nc


def _get_program():
    global _PROGRAM
    if _PROGRAM is None:
        _PROGRAM = _build_program()
    return _PROGRAM


def _make_in_maps(x, W):
    import ml_dtypes

    bf16 = ml_dtypes.bfloat16
    xr = np.ascontiguousarray(x, dtype=np.float32).reshape(N_CORES, RPC, F)
    wpack = np.zeros((128, 128), dtype=bf16)
    wb = np.asarray(W, dtype=np.float32).astype(bf16)
    wpack[0:64, 0:64] = wb
    wpack[64:128, 64:128] = wb
    in_maps = []
    for c in range(N_CORES):
        xpack = np.empty((128, 128 + HALF), dtype=bf16)
        xpack[:, 0:128] = wpack
        xpack[0:64, 128:] = xr[c, 0:HALF].T.astype(bf16)
        xpack[64:128, 128:] = xr[c, HALF:].T.astype(bf16)
        in_maps.append({"xpack": xpack})
    return in_maps


def run_spmd(x, W, **spmd_kwargs):
    """Run the Bass program on 8 cores; returns (y_full, BassKernelResults)."""
    from concourse.bass_utils import run_bass_kernel_spmd

    in_maps = _make_in_maps(x, W)
    res = run_bass_kernel_spmd(
        _get_program(), in_maps, list(range(N_CORES)), **spmd_kwargs
    )
    y = np.empty((N_CORES, RPC, F), np.float32)
    for c in range(N_CORES):
        ypack = np.asarray(res.results[c]["ypack"]).astype(np.float32)
        y[c, 0:HALF] = ypack[0:64].T
        y[c, HALF:] = ypack[64:128].T
    return y.reshape(B, T, N, F), res


def kernel(x, adj, W, a):
    # adj and a are mathematically dead (softmax row-sum == 1); see module doc.
    y, _ = run_spmd(np.asarray(x), np.asarray(W, dtype=np.float32))
    return y
